# revision 28
# baseline (speedup 1.0000x reference)
"""AttentionBlock kernel for 8 Trainium2 NeuronCores.

Reference computation (per batch b):
    Q = x[b] @ Wq + bq            [S, D]
    K = x[b] @ Wk + bk            [S, D]
    V = x[b] @ Wv + bv            [S, D]
    scores = Q @ K^T              [S, S]   (unscaled)
    attn = softmax(scores, -1)
    out[b] = attn @ V / sqrt(D)

Key algebraic restructuring: softmax is invariant to score terms constant
along the key axis, so
    softmax(Q K^T) = softmax(A' x^T),  A' = x_q (Wq Wk^T) + 1 (Wk bq)^T
(M = Wq Wk^T and c = Wk bq are weight-only transforms, computed on host).
This removes the K projection entirely: Q+K projections (384 matmuls per
core incl. a KT DRAM staging round-trip in the first version) become one
A' = x_q M projection (128 matmuls) with c folded in as the ACT eviction
bias. bv passes through softmax (attn rows sum to 1), so V = x @ Wv
unbiased and bv/32 is added to the final output.

Sharding: 8 cores = 4 batches x 2 query-halves. Each core computes V only
for its OWN 1024 query rows (the pair's halves are complementary in
original coordinates) and the halves are exchanged with a pairwise DRAM
AllGather that overlaps the A+scores phases - no duplicated V work. The
kernel is h-agnostic: per-core inputs carry the own-query block as
chunks 0-1 and the full key sequence in ORIGINAL order as chunks 2-5, so
eT s-tiles line up with the gathered V order on every core.

Per-core dataflow (all f32r operands pre-rounded on host to 13 mantissa
bits, matching DVE f32->f32r rounding, and DMA'd straight into resident
f32r SBUF slabs in per-partition-contiguous 16KB lines - no on-device
rounding passes; all matmuls use free-dim 512 so the fp32r LDWEIGHTS
hides under the moving-operand stream):
  1. V_own[s_own, dv] = xq-tiles^T @ Wv (128 mm), evicted bf16, staged to
     DRAM, AllGather -> V full [S, D] bf16, read back during scores.
  2. A' [dk, q] f32r: stationary M-tiles x moving xq columns (128 mm),
     ACT Identity evict with bias c. m chunks stream on the scalar ring
     behind wv, issued after the V matmuls.
  3. scoresT[s-tile, q 512] = xk-tile^T @ A'-block in PSUM (256 mm); Exp
     evict to eT bf16. No max subtraction: max score ~69 stays inside
     f32/bf16 range and softmax is shift-invariant.
  4. rowsum[1, q] via ones(=32)^T @ eT (32 mm; folds the 1/sqrt(d_k));
     reciprocal on DVE; PE-transposed to per-partition [128,1].
  5. attn-output psum[q-tile, dv] = eT^T @ V over 16 s-tiles (256 mm);
     j-groups of 2/2/2/1/1 within 4 PSUM banks while the rowsum banks
     (psr) stay allocated so no bank aliases a late reciprocal; evicted
     with ACT scale=recip and a DVE +bv/32 add.
"""
import sys
from contextlib import ExitStack

sys.path.insert(0, "/opt/trn_rl_repo")

import numpy as np

P = 128
D = 1024            # d_in = d_k = d_v
S = 2048            # kv sequence per core (full batch seq)
NQ = 1024           # query rows per core
B = 4
KT = D // P         # 8 contraction tiles
ST = S // P         # 16 s tiles
QH = 512            # free-dim chunk (fp32r moving-operand limit)
DVC = 512           # dv chunk width

_CACHE = {}


def _build():
    import concourse.bacc as bacc
    import concourse.mybir as mybir
    import concourse.tile as tile

    F32 = mybir.dt.float32
    F32R = mybir.dt.float32r
    BF16 = mybir.dt.bfloat16
    AF = mybir.ActivationFunctionType

    nc = bacc.Bacc("TRN2", target_bir_lowering=False, debug=False, num_devices=8)

    # inputs staged on host in SBUF layout [part, chunk, t, col] so every
    # chunk DMA is 128 descriptors of contiguous 16KB per partition
    xt_d = nc.dram_tensor("xt", [P, 2 + S // QH, KT, QH], F32R, kind="ExternalInput")
    m_d = nc.dram_tensor("m", [P, D // QH, KT, QH], F32R, kind="ExternalInput")
    wv_d = nc.dram_tensor("wv", [P, D // QH, KT, QH], F32R, kind="ExternalInput")
    ct_d = nc.dram_tensor("ct", [P, KT], F32, kind="ExternalInput")
    bvb_d = nc.dram_tensor("bvb", [P, D], mybir.dt.bfloat16, kind="ExternalInput")
    o_d = nc.dram_tensor("o", [NQ, D], F32, kind="ExternalOutput")

    with tile.TileContext(nc) as tc:
        with (
            tc.tile_pool(name="const", bufs=1) as constp,
            tc.tile_pool(name="xrp", bufs=1) as xrp,
            tc.tile_pool(name="ap", bufs=1) as ap_pool,
            tc.tile_pool(name="misc", bufs=1) as miscp,
            tc.tile_pool(name="outp", bufs=4) as outp,
        ):
            ct_sb = constp.tile([P, KT], F32)
            bvb_sb = constp.tile([P, D], BF16)
            ones_f = constp.tile([P, 1], F32)
            nc.vector.memset(ones_f[:], 32.0)
            ones_b = constp.tile([P, 1], BF16)
            nc.vector.tensor_copy(ones_b[:], ones_f[:])
            ident = constp.tile([1, 1], F32)
            nc.vector.memset(ident[:], 1.0)

            xk = xrp.tile([P, S // QH, KT, QH], F32R)  # keys, original order
            A = ap_pool.tile([P, KT, NQ], F32R)    # [dk%128, dk//128, q]

            xq_es = ExitStack()
            xqp = xq_es.enter_context(tc.tile_pool(name="xqp", bufs=1))
            xq = xqp.tile([P, NQ // QH, KT, QH], F32R)  # own query rows
            # sync ring: query chunks first (phase A + V stationaries);
            # chunk 0 in t-halves so the first psum group starts sooner
            nc.sync.dma_start(xq[:, 0, 0:4], xt_d.ap()[:, 0, 0:4])
            nc.sync.dma_start(xq[:, 0, 4:8], xt_d.ap()[:, 0, 4:8])
            nc.sync.dma_start(xq[:, 1], xt_d.ap()[:, 1])

            wvp_es = ExitStack()
            wvp = wvp_es.enter_context(tc.tile_pool(name="wvp", bufs=1))
            proj_es = ExitStack()
            mwp = proj_es.enter_context(tc.tile_pool(name="mw", bufs=1))
            ppp_es = ExitStack()
            ppp = ppp_es.enter_context(
                tc.tile_pool(name="pp", bufs=4, space="PSUM"))

            wv_sb = wvp.tile([P, D // QH, KT, QH], F32R)
            m_sb = mwp.tile([P, D // QH, KT, QH], F32R)
            # scalar ring: ct then m (phase A weights); wv rides the sync
            # ring behind the query chunks so both phases' weights have
            # their doorbells rung at program top on otherwise-idle streams
            nc.scalar.dma_start(ct_sb[:], ct_d.ap())
            nc.scalar.dma_start(m_sb[:, 0, 0:4], m_d.ap()[:, 0, 0:4])
            nc.scalar.dma_start(m_sb[:, 0, 4:8], m_d.ap()[:, 0, 4:8])
            nc.scalar.dma_start(m_sb[:, 1], m_d.ap()[:, 1])
            for c in range(D // QH):
                nc.sync.dma_start(wv_sb[:, c], wv_d.ap()[:, c])
            for c in range(S // QH):
                nc.sync.dma_start(xk[:, c], xt_d.ap()[:, 2 + c])
            nc.scalar.dma_start(bvb_sb[:], bvb_d.ap())

            # ---- A' = x_q @ M + c (ACT bias), 128 matmuls ----
            # dk-halves outer so PE consumption follows the m chunk arrival
            # order (m0 serves both q-chunks before m1 is needed)
            for mh in range(2):
              for qc in range(NQ // QH):
                for dk in range(mh * 4, mh * 4 + 4):
                    ps = ppp.tile([P, QH], F32, tag="pp", name="ps")
                    for t in range(KT):
                        nc.tensor.matmul(
                            ps[:],
                            m_sb[:, dk // 4, t, (dk % 4) * P:(dk % 4 + 1) * P],
                            xq[:, qc, t],
                            start=(t == 0), stop=(t == KT - 1),
                        )
                    nc.scalar.activation(
                        A[:, dk, qc * QH:(qc + 1) * QH], ps[:],
                        AF.Identity, bias=ct_sb[:, dk:dk + 1],
                    )

            proj_es.close()                       # free M before V slabs
            # ---- V for own query rows (the pair's halves are
            # complementary in original coordinates) + pairwise AllGather,
            # which overlaps the scores phase ----
            vo_es = ExitStack()
            vop = vo_es.enter_context(
                tc.tile_pool(name="vop", bufs=1, side="right"))
            dram_es = ExitStack()
            dramp = dram_es.enter_context(
                tc.tile_pool(name="dram", bufs=1, space="DRAM"))
            Vown = vop.tile([P, ST // 2, D], BF16)  # [s%128, s//128, dv]
            v_half = dramp.tile([NQ, D], BF16)
            v_full = dramp.tile([S, D], BF16)
            for dv in range(D // DVC):
                for st in range(ST // 2):
                    ps = ppp.tile([P, DVC], F32, tag="pp", name="ps")
                    for t in range(KT):
                        nc.tensor.matmul(
                            ps[:],
                            xq[:, st // 4, t, (st % 4) * P:(st % 4 + 1) * P],
                            wv_sb[:, dv, t],
                            start=(t == 0), stop=(t == KT - 1),
                        )
                    nc.scalar.copy(Vown[:, st, dv * DVC:(dv + 1) * DVC], ps[:])
            v_half_r = v_half.rearrange("(t p) n -> p t n", p=P)
            nc.gpsimd.dma_start(v_half_r[:], Vown[:])
            nc.gpsimd.collective_compute(
                "AllGather",
                mybir.AluOpType.bypass,
                replica_groups=[[0, 1], [2, 3], [4, 5], [6, 7]],
                ins=[v_half.opt()],
                outs=[v_full.opt()],
            )

            wvp_es.close()                        # free Wv
            ppp_es.close()
            xq_es.close()                         # free query chunks
            vo_es.close()
            vp_es = ExitStack()
            vp = vp_es.enter_context(tc.tile_pool(name="vp", bufs=1, side="right"))
            V = vp.tile([P, ST, D], BF16)          # [s%128, s//128, dv]
            v_full_r = v_full.rearrange("(t p) n -> p t n", p=P)
            # readback overlaps the scores phase on the sync ring
            for t2 in range(2):
                nc.sync.dma_start(V[:, t2 * 8:(t2 + 1) * 8],
                                  v_full_r[:, t2 * 8:(t2 + 1) * 8])

            # ---- attention ----
            etp_es = ExitStack()
            etp = etp_es.enter_context(tc.tile_pool(name="etp", bufs=1, side="right"))
            eT = etp.tile([P, ST, NQ], BF16, tag="eT", name="eT")
            psr_es = ExitStack()
            psr = psr_es.enter_context(
                tc.tile_pool(name="psr", bufs=2, space="PSUM"))
            scr_es = ExitStack()
            pss = scr_es.enter_context(
                tc.tile_pool(name="pss", bufs=2, space="PSUM"))
            # scoresT[s-tile, q] = xT-tile^T @ A'-block, accumulated over
            # dk; rowsum as one contiguous bf16 block afterwards (mixing the
            # bf16 ones-matmuls into the f32r stream costs a PE mode switch
            # every 8 matmuls)
            for st in range(ST):
                for qh in range(NQ // QH):
                    ps = pss.tile([P, QH], F32, tag="ps", name="ps")
                    for t in range(KT):
                        nc.tensor.matmul(
                            ps[:],
                            xk[:, st // 4, t, (st % 4) * P:(st % 4 + 1) * P],
                            A[:, t, qh * QH:(qh + 1) * QH],
                            start=(t == 0), stop=(t == KT - 1),
                        )
                    nc.scalar.activation(
                        eT[:, st, qh * QH:(qh + 1) * QH], ps[:], AF.Exp)
            rec32s = []
            for qh in range(NQ // QH):
                prs = psr.tile([1, QH], F32, tag="prs", name="prs")
                for st in range(ST):
                    nc.tensor.matmul(
                        prs[:], ones_b[:], eT[:, st, qh * QH:(qh + 1) * QH],
                        start=(st == 0), stop=(st == ST - 1))
                rec32 = miscp.tile([1, QH], F32, tag=f"rec32{qh}", name="rec32")
                nc.vector.reciprocal(rec32[:], prs[:])
                rec32s.append(rec32)
            scr_es.close()

            # attn @ V in j-groups of 3/3/1/1 (6 PSUM banks max, small tail)
            with (
                tc.tile_pool(name="pso", bufs=1, space="PSUM") as pso,
                tc.tile_pool(name="pst", bufs=1, space="PSUM") as pst,
            ):
                rcs = []
                groups = [(0, 1), (2, 3), (4, 5), (6,), (7,)]
                for gi, js in enumerate(groups):
                    pos = [
                        pso.tile([P, DVC], F32, tag=f"po{u}", name="po")
                        for u in range(len(js) * (D // DVC))
                    ]
                    for ji, j in enumerate(js):
                        for dv in range(D // DVC):
                            for st in range(ST):
                                nc.tensor.matmul(
                                    pos[ji * (D // DVC) + dv][:],
                                    eT[:, st, j * P:(j + 1) * P],
                                    V[:, st, dv * DVC:(dv + 1) * DVC],
                                    start=(st == 0), stop=(st == ST - 1),
                                )
                    if gi == 0:
                        # emitted after a dense MM batch so the ACT->DVE->PE
                        # reciprocal/transpose chain hides under the matmuls
                        for j in range(NQ // P):
                            qh, jq = divmod(j, QH // P)
                            pt = pst.tile([P, 1], F32, tag="pt", name="pt")
                            nc.tensor.transpose(
                                pt[:], rec32s[qh][:, jq * P:(jq + 1) * P],
                                ident[:])
                            rc = miscp.tile([P, 1], F32, tag=f"rc{j}", name="rc")
                            # 1/sqrt(d_k) is folded into ones=32 upstream
                            nc.vector.tensor_copy(rc[:], pt[:])
                            rcs.append(rc)
                    for ji, j in enumerate(js):
                        for dv in range(D // DVC):
                            po = pos[ji * (D // DVC) + dv]
                            osb = outp.tile([P, DVC], F32, tag="osb", name="osb")
                            nc.scalar.activation(osb[:], po[:], AF.Copy,
                                                 scale=rcs[j][:])
                            nc.vector.tensor_tensor(
                                osb[:], osb[:],
                                bvb_sb[:, dv * DVC:(dv + 1) * DVC],
                                op=mybir.AluOpType.add,
                            )
                            nc.scalar.dma_start(
                                o_d.ap()[j * P:(j + 1) * P,
                                         dv * DVC:(dv + 1) * DVC],
                                osb[:],
                            )
            psr_es.close()
            etp_es.close()
            vp_es.close()
            dram_es.close()
    nc.compile()
    return nc


def _get_nc():
    if "nc" not in _CACHE:
        _CACHE["nc"] = _build()
    return _CACHE["nc"]


def _preround(a, bits=13):
    # round mantissa to `bits` explicit bits (round-to-nearest), matching
    # the DVE f32->f32r rounding so raw DMA into f32r tiles is faithful
    u = np.ascontiguousarray(a, dtype=np.float32).view(np.uint32)
    shift = 23 - bits
    add = np.uint32(1 << (shift - 1))
    u = ((u.astype(np.uint64) + add) >> shift << shift).astype(np.uint32)
    return np.ascontiguousarray(u.view(np.float32))


def _in_maps(x, Wq, bq, Wk, bk, Wv, bv):
    import ml_dtypes
    def _stage(w):
        # [D, N] -> [128, N//512, 8, 512]: per-partition contiguous chunks
        return np.ascontiguousarray(
            w.reshape(KT, P, -1, QH).transpose(1, 2, 0, 3))

    M = _stage(_preround(
        np.asarray(Wq, np.float64) @ np.asarray(Wk, np.float64).T))
    c = (np.asarray(Wk, np.float64) @ np.asarray(bq, np.float64)).astype(np.float32)
    ct = np.ascontiguousarray(np.reshape(c, (KT, P)).T, dtype=np.float32)
    wv = _stage(_preround(Wv))
    bvb = np.ascontiguousarray(
        np.tile(np.asarray(bv, np.float32) / 32.0, (P, 1)).astype(ml_dtypes.bfloat16))
    x = np.asarray(x, np.float32)
    xk_stage = [_stage(_preround(x[b].T)) for b in range(B)]
    maps = []
    for cidx in range(8):
        b, h = cidx // 2, cidx % 2
        # chunks 0-1: own query rows; chunks 2-5: full x, original order
        xq = _stage(_preround(x[b, h * NQ:(h + 1) * NQ].T))
        xt = np.ascontiguousarray(np.concatenate([xq, xk_stage[b]], axis=1))
        maps.append({"xt": xt, "m": M, "wv": wv, "ct": ct, "bvb": bvb})
    return maps


def _run(inputs, trace=False, tmpdir=None):
    import time

    from concourse.bass_utils import run_bass_kernel_spmd

    nc = _get_nc()
    maps = _in_maps(**inputs)
    last_err = None
    for attempt in range(3):
        try:
            res = run_bass_kernel_spmd(nc, maps, core_ids=list(range(8)),
                                       trace=trace, tmpdir=tmpdir)
            break
        except Exception as e:  # transient NRT device errors recover on retry
            last_err = e
            time.sleep(10)
    else:
        raise last_err
    out = np.empty((B, 2 * NQ, D), dtype=np.float32)
    for cidx in range(8):
        b, h = cidx // 2, cidx % 2
        out[b, h * NQ:(h + 1) * NQ, :] = res.results[cidx]["o"]
    return out, res


def kernel(**inputs):
    out, _ = _run(inputs, trace=False)
    return out


# revision 29
# speedup vs baseline: 1.0764x; 1.0764x over previous
"""AttentionBlock kernel for 8 Trainium2 NeuronCores.

Reference computation (per batch b):
    Q = x[b] @ Wq + bq            [S, D]
    K = x[b] @ Wk + bk            [S, D]
    V = x[b] @ Wv + bv            [S, D]
    scores = Q @ K^T              [S, S]   (unscaled)
    attn = softmax(scores, -1)
    out[b] = attn @ V / sqrt(D)

Key algebraic restructuring: softmax is invariant to score terms constant
along the key axis, so
    softmax(Q K^T) = softmax(A' x^T),  A' = x_q (Wq Wk^T) + 1 (Wk bq)^T
(M = Wq Wk^T and c = Wk bq are weight-only transforms, computed on host).
This removes the K projection entirely: Q+K projections (384 matmuls per
core incl. a KT DRAM staging round-trip in the first version) become one
A' = x_q M projection (128 matmuls) with c folded in as the ACT eviction
bias. bv passes through softmax (attn rows sum to 1), so V = x @ Wv
unbiased and bv/32 is added to the final output.

Sharding: 8 cores = 4 batches x 2 query-halves. Each core computes V only
for its OWN 1024 query rows (the pair's halves are complementary in
original coordinates) and the halves are exchanged with a pairwise DRAM
AllGather that overlaps the A+scores phases - no duplicated V work. The
kernel is h-agnostic: per-core inputs carry the own-query block as
chunks 0-1 and the full key sequence in ORIGINAL order as chunks 2-5, so
eT s-tiles line up with the gathered V order on every core.

Per-core dataflow (all f32r operands pre-rounded on host to 13 mantissa
bits, matching DVE f32->f32r rounding, and DMA'd straight into resident
f32r SBUF slabs in per-partition-contiguous 16KB lines - no on-device
rounding passes; all matmuls use free-dim 512 so the fp32r LDWEIGHTS
hides under the moving-operand stream):
  1. A' [dk, q] f32r: stationary M-tiles x moving xq columns (128 mm),
     ACT Identity evict with bias c. m rides the scalar ring, wv rides
     the sync ring behind xq - both doorbells ring at program top on
     otherwise-idle engine streams, first chunks split in t-halves so
     the first psum group starts as early as possible.
  2. V_own[s_own, dv] = xq-tiles^T @ Wv (128 mm), evicted bf16, staged to
     DRAM, AllGather -> V full [S, D] bf16, read back during scores.
  3. scoresT[s-tile, q 512] = xk-tile^T @ A'-block in PSUM (256 mm); Exp
     evict to eT bf16. No max subtraction: max score ~69 stays inside
     f32/bf16 range and softmax is shift-invariant.
  4. rowsum[1, q] via ones(=32)^T @ eT (32 mm; folds the 1/sqrt(d_k));
     reciprocal on DVE; PE-transposed to per-partition [128,1].
  5. attn-output psum[q-tile, dv] = eT^T @ V over 16 s-tiles (256 mm);
     j-groups of 2/2/2/1/1 within 4 PSUM banks while the rowsum banks
     (psr) stay allocated so no bank aliases a late reciprocal; evicted
     with ACT scale=recip and a DVE +bv/32 add.
"""
import sys
from contextlib import ExitStack

sys.path.insert(0, "/opt/trn_rl_repo")

import numpy as np

P = 128
D = 1024            # d_in = d_k = d_v
S = 2048            # kv sequence per core (full batch seq)
NQ = 1024           # query rows per core
B = 4
KT = D // P         # 8 contraction tiles
ST = S // P         # 16 s tiles
QH = 512            # free-dim chunk (fp32r moving-operand limit)
DVC = 512           # dv chunk width

_CACHE = {}


def _build():
    import concourse.bacc as bacc
    import concourse.mybir as mybir
    import concourse.tile as tile

    F32 = mybir.dt.float32
    F32R = mybir.dt.float32r
    BF16 = mybir.dt.bfloat16
    AF = mybir.ActivationFunctionType

    nc = bacc.Bacc("TRN2", target_bir_lowering=False, debug=False, num_devices=8)

    # inputs staged on host in SBUF layout [part, chunk, t, col] so every
    # chunk DMA is 128 descriptors of contiguous 16KB per partition
    xt_d = nc.dram_tensor("xt", [P, 2 + S // QH, KT, QH], F32R, kind="ExternalInput")
    m_d = nc.dram_tensor("m", [P, D // QH, KT, QH], F32R, kind="ExternalInput")
    wv_d = nc.dram_tensor("wv", [P, D // QH, KT, QH], F32R, kind="ExternalInput")
    ct_d = nc.dram_tensor("ct", [P, KT], F32, kind="ExternalInput")
    bvb_d = nc.dram_tensor("bvb", [P, D], mybir.dt.bfloat16, kind="ExternalInput")
    o_d = nc.dram_tensor("o", [NQ, D], F32, kind="ExternalOutput")

    with tile.TileContext(nc) as tc:
        with (
            tc.tile_pool(name="const", bufs=1) as constp,
            tc.tile_pool(name="xrp", bufs=1) as xrp,
            tc.tile_pool(name="ap", bufs=1) as ap_pool,
            tc.tile_pool(name="misc", bufs=1) as miscp,
            tc.tile_pool(name="outp", bufs=4) as outp,
        ):
            ct_sb = constp.tile([P, KT], F32)
            bvb_sb = constp.tile([P, D], BF16)
            ones_f = constp.tile([P, 1], F32)
            nc.vector.memset(ones_f[:], 32.0)
            ones_b = constp.tile([P, 1], BF16)
            nc.vector.tensor_copy(ones_b[:], ones_f[:])
            ident = constp.tile([1, 1], F32)
            nc.vector.memset(ident[:], 1.0)

            xk = xrp.tile([P, S // QH, KT, QH], F32R)  # keys, original order
            A = ap_pool.tile([P, KT, NQ], F32R)    # [dk%128, dk//128, q]

            xq_es = ExitStack()
            xqp = xq_es.enter_context(tc.tile_pool(name="xqp", bufs=1))
            xq = xqp.tile([P, NQ // QH, KT, QH], F32R)  # own query rows
            # sync ring: query chunks first (phase A + V stationaries);
            # chunk 0 in t-halves so the first psum group starts sooner
            nc.sync.dma_start(xq[:, 0, 0:4], xt_d.ap()[:, 0, 0:4])
            nc.sync.dma_start(xq[:, 0, 4:8], xt_d.ap()[:, 0, 4:8])
            nc.sync.dma_start(xq[:, 1], xt_d.ap()[:, 1])

            wvp_es = ExitStack()
            wvp = wvp_es.enter_context(tc.tile_pool(name="wvp", bufs=1))
            proj_es = ExitStack()
            mwp = proj_es.enter_context(tc.tile_pool(name="mw", bufs=1))
            ppp_es = ExitStack()
            ppp = ppp_es.enter_context(
                tc.tile_pool(name="pp", bufs=4, space="PSUM"))

            wv_sb = wvp.tile([P, D // QH, KT, QH], F32R)
            m_sb = mwp.tile([P, D // QH, KT, QH], F32R)
            # scalar ring: ct then m (phase A weights); wv rides the sync
            # ring behind the query chunks so both phases' weights have
            # their doorbells rung at program top on otherwise-idle streams
            nc.scalar.dma_start(ct_sb[:], ct_d.ap())
            nc.scalar.dma_start(m_sb[:, 0, 0:4], m_d.ap()[:, 0, 0:4])
            nc.scalar.dma_start(m_sb[:, 0, 4:8], m_d.ap()[:, 0, 4:8])
            nc.scalar.dma_start(m_sb[:, 1], m_d.ap()[:, 1])
            for c in range(D // QH):
                nc.sync.dma_start(wv_sb[:, c], wv_d.ap()[:, c])
            for c in range(S // QH):
                nc.sync.dma_start(xk[:, c], xt_d.ap()[:, 2 + c])
            nc.scalar.dma_start(bvb_sb[:], bvb_d.ap())

            # ---- A' = x_q @ M + c (ACT bias), 128 matmuls ----
            # dk-halves outer so PE consumption follows the m chunk arrival
            # order (m0 serves both q-chunks before m1 is needed)
            for mh in range(2):
              for qc in range(NQ // QH):
                for dk in range(mh * 4, mh * 4 + 4):
                    ps = ppp.tile([P, QH], F32, tag="pp", name="ps")
                    for t in range(KT):
                        nc.tensor.matmul(
                            ps[:],
                            m_sb[:, dk // 4, t, (dk % 4) * P:(dk % 4 + 1) * P],
                            xq[:, qc, t],
                            start=(t == 0), stop=(t == KT - 1),
                        )
                    nc.scalar.activation(
                        A[:, dk, qc * QH:(qc + 1) * QH], ps[:],
                        AF.Identity, bias=ct_sb[:, dk:dk + 1],
                    )

            proj_es.close()                       # free M before V slabs
            # ---- V for own query rows (the pair's halves are
            # complementary in original coordinates) + pairwise AllGather,
            # which overlaps the scores phase ----
            vo_es = ExitStack()
            vop = vo_es.enter_context(
                tc.tile_pool(name="vop", bufs=1, side="right"))
            dram_es = ExitStack()
            dramp = dram_es.enter_context(
                tc.tile_pool(name="dram", bufs=1, space="DRAM"))
            Vown = vop.tile([P, ST // 2, D], BF16)  # [s%128, s//128, dv]
            v_half = dramp.tile([NQ, D], BF16)
            v_full = dramp.tile([S, D], BF16)
            for dv in range(D // DVC):
                for st in range(ST // 2):
                    ps = ppp.tile([P, DVC], F32, tag="pp", name="ps")
                    for t in range(KT):
                        nc.tensor.matmul(
                            ps[:],
                            xq[:, st // 4, t, (st % 4) * P:(st % 4 + 1) * P],
                            wv_sb[:, dv, t],
                            start=(t == 0), stop=(t == KT - 1),
                        )
                    nc.scalar.copy(Vown[:, st, dv * DVC:(dv + 1) * DVC], ps[:])
            v_half_r = v_half.rearrange("(t p) n -> p t n", p=P)
            nc.gpsimd.dma_start(v_half_r[:], Vown[:])
            nc.gpsimd.collective_compute(
                "AllGather",
                mybir.AluOpType.bypass,
                replica_groups=[[0, 1], [2, 3], [4, 5], [6, 7]],
                ins=[v_half.opt()],
                outs=[v_full.opt()],
            )

            wvp_es.close()                        # free Wv
            ppp_es.close()
            xq_es.close()                         # free query chunks
            vo_es.close()
            vp_es = ExitStack()
            vp = vp_es.enter_context(tc.tile_pool(name="vp", bufs=1, side="right"))
            V = vp.tile([P, ST, D], BF16)          # [s%128, s//128, dv]
            v_full_r = v_full.rearrange("(t p) n -> p t n", p=P)
            # readback overlaps the scores phase on the sync ring
            for t2 in range(2):
                nc.sync.dma_start(V[:, t2 * 8:(t2 + 1) * 8],
                                  v_full_r[:, t2 * 8:(t2 + 1) * 8])

            # ---- attention ----
            etp_es = ExitStack()
            etp = etp_es.enter_context(tc.tile_pool(name="etp", bufs=1, side="right"))
            eT = etp.tile([P, ST, NQ], BF16, tag="eT", name="eT")
            psr_es = ExitStack()
            psr = psr_es.enter_context(
                tc.tile_pool(name="psr", bufs=2, space="PSUM"))
            scr_es = ExitStack()
            pss = scr_es.enter_context(
                tc.tile_pool(name="pss", bufs=2, space="PSUM"))
            # scoresT[s-tile, q] = xT-tile^T @ A'-block, accumulated over
            # dk; rowsum as one contiguous bf16 block afterwards (mixing the
            # bf16 ones-matmuls into the f32r stream costs a PE mode switch
            # every 8 matmuls)
            for st in range(ST):
                for qh in range(NQ // QH):
                    ps = pss.tile([P, QH], F32, tag="ps", name="ps")
                    for t in range(KT):
                        nc.tensor.matmul(
                            ps[:],
                            xk[:, st // 4, t, (st % 4) * P:(st % 4 + 1) * P],
                            A[:, t, qh * QH:(qh + 1) * QH],
                            start=(t == 0), stop=(t == KT - 1),
                        )
                    nc.scalar.activation(
                        eT[:, st, qh * QH:(qh + 1) * QH], ps[:], AF.Exp)
            rec32s = []
            for qh in range(NQ // QH):
                prs = psr.tile([1, QH], F32, tag="prs", name="prs")
                for st in range(ST):
                    nc.tensor.matmul(
                        prs[:], ones_b[:], eT[:, st, qh * QH:(qh + 1) * QH],
                        start=(st == 0), stop=(st == ST - 1))
                rec32 = miscp.tile([1, QH], F32, tag=f"rec32{qh}", name="rec32")
                nc.vector.reciprocal(rec32[:], prs[:])
                rec32s.append(rec32)
            scr_es.close()

            # attn @ V in j-groups of 3/3/1/1 (6 PSUM banks max, small tail)
            with (
                tc.tile_pool(name="pso", bufs=1, space="PSUM") as pso,
                tc.tile_pool(name="pst", bufs=1, space="PSUM") as pst,
            ):
                rcs = []
                groups = [(0, 1), (2, 3), (4, 5), (6,), (7,)]
                for gi, js in enumerate(groups):
                    pos = [
                        pso.tile([P, DVC], F32, tag=f"po{u}", name="po")
                        for u in range(len(js) * (D // DVC))
                    ]
                    for ji, j in enumerate(js):
                        for dv in range(D // DVC):
                            for st in range(ST):
                                nc.tensor.matmul(
                                    pos[ji * (D // DVC) + dv][:],
                                    eT[:, st, j * P:(j + 1) * P],
                                    V[:, st, dv * DVC:(dv + 1) * DVC],
                                    start=(st == 0), stop=(st == ST - 1),
                                )
                    if gi == 0:
                        # emitted after a dense MM batch so the ACT->DVE->PE
                        # reciprocal/transpose chain hides under the matmuls
                        for j in range(NQ // P):
                            qh, jq = divmod(j, QH // P)
                            pt = pst.tile([P, 1], F32, tag="pt", name="pt")
                            nc.tensor.transpose(
                                pt[:], rec32s[qh][:, jq * P:(jq + 1) * P],
                                ident[:])
                            rc = miscp.tile([P, 1], F32, tag=f"rc{j}", name="rc")
                            # 1/sqrt(d_k) is folded into ones=32 upstream
                            nc.vector.tensor_copy(rc[:], pt[:])
                            rcs.append(rc)
                    for ji, j in enumerate(js):
                        for dv in range(D // DVC):
                            po = pos[ji * (D // DVC) + dv]
                            osb = outp.tile([P, DVC], F32, tag="osb", name="osb")
                            nc.scalar.activation(osb[:], po[:], AF.Copy,
                                                 scale=rcs[j][:])
                            nc.vector.tensor_tensor(
                                osb[:], osb[:],
                                bvb_sb[:, dv * DVC:(dv + 1) * DVC],
                                op=mybir.AluOpType.add,
                            )
                            nc.scalar.dma_start(
                                o_d.ap()[j * P:(j + 1) * P,
                                         dv * DVC:(dv + 1) * DVC],
                                osb[:],
                            )
            psr_es.close()
            etp_es.close()
            vp_es.close()
            dram_es.close()
    nc.compile()
    return nc


def _get_nc():
    if "nc" not in _CACHE:
        _CACHE["nc"] = _build()
    return _CACHE["nc"]


def _preround(a, bits=13):
    # round mantissa to `bits` explicit bits (round-to-nearest), matching
    # the DVE f32->f32r rounding so raw DMA into f32r tiles is faithful
    u = np.ascontiguousarray(a, dtype=np.float32).view(np.uint32)
    shift = 23 - bits
    add = np.uint32(1 << (shift - 1))
    u = ((u.astype(np.uint64) + add) >> shift << shift).astype(np.uint32)
    return np.ascontiguousarray(u.view(np.float32))


def _in_maps(x, Wq, bq, Wk, bk, Wv, bv):
    import ml_dtypes
    def _stage(w):
        # [D, N] -> [128, N//512, 8, 512]: per-partition contiguous chunks
        return np.ascontiguousarray(
            w.reshape(KT, P, -1, QH).transpose(1, 2, 0, 3))

    M = _stage(_preround(
        np.asarray(Wq, np.float64) @ np.asarray(Wk, np.float64).T))
    c = (np.asarray(Wk, np.float64) @ np.asarray(bq, np.float64)).astype(np.float32)
    ct = np.ascontiguousarray(np.reshape(c, (KT, P)).T, dtype=np.float32)
    wv = _stage(_preround(Wv))
    bvb = np.ascontiguousarray(
        np.tile(np.asarray(bv, np.float32) / 32.0, (P, 1)).astype(ml_dtypes.bfloat16))
    x = np.asarray(x, np.float32)
    xk_stage = [_stage(_preround(x[b].T)) for b in range(B)]
    maps = []
    for cidx in range(8):
        b, h = cidx // 2, cidx % 2
        # chunks 0-1: own query rows; chunks 2-5: full x, original order
        xq = _stage(_preround(x[b, h * NQ:(h + 1) * NQ].T))
        xt = np.ascontiguousarray(np.concatenate([xq, xk_stage[b]], axis=1))
        maps.append({"xt": xt, "m": M, "wv": wv, "ct": ct, "bvb": bvb})
    return maps


def _run(inputs, trace=False, tmpdir=None):
    import time

    from concourse.bass_utils import run_bass_kernel_spmd

    nc = _get_nc()
    maps = _in_maps(**inputs)
    last_err = None
    for attempt in range(3):
        try:
            res = run_bass_kernel_spmd(nc, maps, core_ids=list(range(8)),
                                       trace=trace, tmpdir=tmpdir)
            break
        except Exception as e:  # transient NRT device errors recover on retry
            last_err = e
            time.sleep(10)
    else:
        raise last_err
    out = np.empty((B, 2 * NQ, D), dtype=np.float32)
    for cidx in range(8):
        b, h = cidx // 2, cidx % 2
        out[b, h * NQ:(h + 1) * NQ, :] = res.results[cidx]["o"]
    return out, res


def kernel(**inputs):
    out, _ = _run(inputs, trace=False)
    return out


# revision 30
# speedup vs baseline: 1.0860x; 1.0088x over previous
"""AttentionBlock kernel for 8 Trainium2 NeuronCores.

Reference computation (per batch b):
    Q = x[b] @ Wq + bq            [S, D]
    K = x[b] @ Wk + bk            [S, D]
    V = x[b] @ Wv + bv            [S, D]
    scores = Q @ K^T              [S, S]   (unscaled)
    attn = softmax(scores, -1)
    out[b] = attn @ V / sqrt(D)

Key algebraic restructuring: softmax is invariant to score terms constant
along the key axis, so
    softmax(Q K^T) = softmax(A' x^T),  A' = x_q (Wq Wk^T) + 1 (Wk bq)^T
(M = Wq Wk^T and c = Wk bq are weight-only transforms, computed on host).
This removes the K projection entirely: Q+K projections (384 matmuls per
core incl. a KT DRAM staging round-trip in the first version) become one
A' = x_q M projection (128 matmuls) with c folded in as the ACT eviction
bias. bv passes through softmax (attn rows sum to 1), so V = x @ Wv
unbiased and bv/32 is added to the final output.

Sharding: 8 cores = 4 batches x 2 query-halves. Each core computes V only
for its OWN 1024 query rows (the pair's halves are complementary in
original coordinates) and the halves are exchanged with a pairwise DRAM
AllGather that overlaps the A+scores phases - no duplicated V work. The
kernel is h-agnostic: per-core inputs carry the own-query block as
chunks 0-1 and the full key sequence in ORIGINAL order as chunks 2-5, so
eT s-tiles line up with the gathered V order on every core.

Per-core dataflow (all f32r operands pre-rounded on host to 13 mantissa
bits, matching DVE f32->f32r rounding, and DMA'd straight into resident
f32r SBUF slabs in per-partition-contiguous 16KB lines - no on-device
rounding passes; all matmuls use free-dim 512 so the fp32r LDWEIGHTS
hides under the moving-operand stream):
  1. A' [dk, q] f32r: stationary M-tiles x moving xq columns (128 mm),
     ACT Identity evict with bias c. m rides the scalar ring, wv rides
     the sync ring behind xq - both doorbells ring at program top on
     otherwise-idle engine streams, first chunks split in t-halves so
     the first psum group starts as early as possible.
  2. V_own[s_own, dv] = xq-tiles^T @ Wv (128 mm), evicted bf16, staged to
     DRAM, AllGather -> V full [S, D] bf16, read back during scores.
  3. scoresT[s-tile, q 512] = xk-tile^T @ A'-block in PSUM (256 mm); Exp
     evict to eT bf16. No max subtraction: max score ~69 stays inside
     f32/bf16 range and softmax is shift-invariant.
  4. rowsum[1, q] via ones(=32)^T @ eT (32 mm; folds the 1/sqrt(d_k));
     reciprocal on DVE; PE-transposed to per-partition [128,1].
  5. attn-output psum[q-tile, dv] = eT^T @ V over 16 s-tiles (256 mm);
     j-groups of 2/2/2/1/1 within 4 PSUM banks while the rowsum banks
     (psr) stay allocated so no bank aliases a late reciprocal; evicted
     with ACT scale=recip and a DVE +bv/32 add.
"""
import sys
from contextlib import ExitStack

sys.path.insert(0, "/opt/trn_rl_repo")

import numpy as np

P = 128
D = 1024            # d_in = d_k = d_v
S = 2048            # kv sequence per core (full batch seq)
NQ = 1024           # query rows per core
B = 4
KT = D // P         # 8 contraction tiles
ST = S // P         # 16 s tiles
QH = 512            # free-dim chunk (fp32r moving-operand limit)
DVC = 512           # dv chunk width

_CACHE = {}


def _build():
    import concourse.bacc as bacc
    import concourse.mybir as mybir
    import concourse.tile as tile

    F32 = mybir.dt.float32
    F32R = mybir.dt.float32r
    BF16 = mybir.dt.bfloat16
    AF = mybir.ActivationFunctionType

    nc = bacc.Bacc("TRN2", target_bir_lowering=False, debug=False, num_devices=8)

    # inputs staged on host in SBUF layout [part, chunk, t, col] so every
    # chunk DMA is 128 descriptors of contiguous 16KB per partition
    xt_d = nc.dram_tensor("xt", [P, 2 + S // QH, KT, QH], F32R, kind="ExternalInput")
    m_d = nc.dram_tensor("m", [P, D // QH, KT, QH], F32R, kind="ExternalInput")
    wv_d = nc.dram_tensor("wv", [P, D // QH, KT, QH], F32R, kind="ExternalInput")
    ct_d = nc.dram_tensor("ct", [P, KT], F32, kind="ExternalInput")
    bvb_d = nc.dram_tensor("bvb", [P, D], mybir.dt.bfloat16, kind="ExternalInput")
    o_d = nc.dram_tensor("o", [NQ, D], F32, kind="ExternalOutput")

    with tile.TileContext(nc) as tc:
        with (
            tc.tile_pool(name="const", bufs=1) as constp,
            tc.tile_pool(name="xrp", bufs=1) as xrp,
            tc.tile_pool(name="ap", bufs=1) as ap_pool,
            tc.tile_pool(name="misc", bufs=1) as miscp,
            tc.tile_pool(name="outp", bufs=4) as outp,
        ):
            ct_sb = constp.tile([P, KT], F32)
            bvb_sb = constp.tile([P, D], BF16)
            ones_f = constp.tile([P, 1], F32)
            nc.vector.memset(ones_f[:], 32.0)
            ones_b = constp.tile([P, 1], BF16)
            nc.vector.tensor_copy(ones_b[:], ones_f[:])
            ident = constp.tile([1, 1], F32)
            nc.vector.memset(ident[:], 1.0)

            xk = xrp.tile([P, S // QH, KT, QH], F32R)  # keys, original order
            A = ap_pool.tile([P, KT, NQ], F32R)    # [dk%128, dk//128, q]

            xq_es = ExitStack()
            xqp = xq_es.enter_context(tc.tile_pool(name="xqp", bufs=1))
            xq = xqp.tile([P, NQ // QH, KT, QH], F32R)  # own query rows
            # sync ring: query chunks first (phase A + V stationaries);
            # chunk 0 in t-halves so the first psum group starts sooner
            nc.sync.dma_start(xq[:, 0, 0:4], xt_d.ap()[:, 0, 0:4])
            nc.sync.dma_start(xq[:, 0, 4:8], xt_d.ap()[:, 0, 4:8])
            nc.sync.dma_start(xq[:, 1], xt_d.ap()[:, 1])

            wvp_es = ExitStack()
            wvp = wvp_es.enter_context(tc.tile_pool(name="wvp", bufs=1))
            proj_es = ExitStack()
            mwp = proj_es.enter_context(tc.tile_pool(name="mw", bufs=1))
            ppp_es = ExitStack()
            ppp = ppp_es.enter_context(
                tc.tile_pool(name="pp", bufs=4, space="PSUM"))

            wv_sb = wvp.tile([P, D // QH, KT, QH], F32R)
            m_sb = mwp.tile([P, D // QH, KT, QH], F32R)
            # scalar ring: ct then m (phase A weights); wv rides the sync
            # ring behind the query chunks so both phases' weights have
            # their doorbells rung at program top on otherwise-idle streams
            nc.scalar.dma_start(ct_sb[:], ct_d.ap())
            nc.scalar.dma_start(m_sb[:, 0, 0:4], m_d.ap()[:, 0, 0:4])
            nc.scalar.dma_start(m_sb[:, 0, 4:8], m_d.ap()[:, 0, 4:8])
            nc.scalar.dma_start(m_sb[:, 1], m_d.ap()[:, 1])
            for c in range(D // QH):
                nc.sync.dma_start(wv_sb[:, c], wv_d.ap()[:, c])
            for c in range(S // QH):
                nc.sync.dma_start(xk[:, c], xt_d.ap()[:, 2 + c])
            nc.scalar.dma_start(bvb_sb[:], bvb_d.ap())

            # ---- A' = x_q @ M + c (ACT bias), 128 matmuls ----
            # dk-halves outer so PE consumption follows the m chunk arrival
            # order (m0 serves both q-chunks before m1 is needed)
            for mh in range(2):
              for qc in range(NQ // QH):
                for dk in range(mh * 4, mh * 4 + 4):
                    ps = ppp.tile([P, QH], F32, tag="pp", name="ps")
                    for t in range(KT):
                        nc.tensor.matmul(
                            ps[:],
                            m_sb[:, dk // 4, t, (dk % 4) * P:(dk % 4 + 1) * P],
                            xq[:, qc, t],
                            start=(t == 0), stop=(t == KT - 1),
                        )
                    nc.scalar.activation(
                        A[:, dk, qc * QH:(qc + 1) * QH], ps[:],
                        AF.Identity, bias=ct_sb[:, dk:dk + 1],
                    )

            proj_es.close()                       # free M before V slabs
            # ---- V for own query rows (the pair's halves are
            # complementary in original coordinates) + pairwise AllGather,
            # which overlaps the scores phase ----
            vo_es = ExitStack()
            vop = vo_es.enter_context(
                tc.tile_pool(name="vop", bufs=1, side="right"))
            dram_es = ExitStack()
            dramp = dram_es.enter_context(
                tc.tile_pool(name="dram", bufs=1, space="DRAM"))
            Vown = vop.tile([P, ST // 2, D], BF16)  # [s%128, s//128, dv]
            v_half = dramp.tile([NQ, D], BF16)
            v_full = dramp.tile([S, D], BF16)
            for dv in range(D // DVC):
                for st in range(ST // 2):
                    ps = ppp.tile([P, DVC], F32, tag="pp", name="ps")
                    for t in range(KT):
                        nc.tensor.matmul(
                            ps[:],
                            xq[:, st // 4, t, (st % 4) * P:(st % 4 + 1) * P],
                            wv_sb[:, dv, t],
                            start=(t == 0), stop=(t == KT - 1),
                        )
                    nc.scalar.copy(Vown[:, st, dv * DVC:(dv + 1) * DVC], ps[:])
            v_half_r = v_half.rearrange("(t p) n -> p t n", p=P)
            nc.gpsimd.dma_start(v_half_r[:], Vown[:])
            nc.gpsimd.collective_compute(
                "AllGather",
                mybir.AluOpType.bypass,
                replica_groups=[[0, 1], [2, 3], [4, 5], [6, 7]],
                ins=[v_half.opt()],
                outs=[v_full.opt()],
            )

            wvp_es.close()                        # free Wv
            ppp_es.close()
            xq_es.close()                         # free query chunks
            vo_es.close()
            vp_es = ExitStack()
            vp = vp_es.enter_context(tc.tile_pool(name="vp", bufs=1, side="right"))
            V = vp.tile([P, ST, D], BF16)          # [s%128, s//128, dv]
            v_full_r = v_full.rearrange("(t p) n -> p t n", p=P)
            # readback overlaps the scores phase on the sync ring
            for t2 in range(2):
                nc.sync.dma_start(V[:, t2 * 8:(t2 + 1) * 8],
                                  v_full_r[:, t2 * 8:(t2 + 1) * 8])

            # ---- attention ----
            etp_es = ExitStack()
            etp = etp_es.enter_context(tc.tile_pool(name="etp", bufs=1, side="right"))
            eT = etp.tile([P, ST, NQ], BF16, tag="eT", name="eT")
            psr_es = ExitStack()
            psr = psr_es.enter_context(
                tc.tile_pool(name="psr", bufs=2, space="PSUM"))
            scr_es = ExitStack()
            pss = scr_es.enter_context(
                tc.tile_pool(name="pss", bufs=2, space="PSUM"))
            # scoresT[s-tile, q] = xT-tile^T @ A'-block, accumulated over
            # dk; rowsum as one contiguous bf16 block afterwards (mixing the
            # bf16 ones-matmuls into the f32r stream costs a PE mode switch
            # every 8 matmuls)
            for st in range(ST):
                for qh in range(NQ // QH):
                    ps = pss.tile([P, QH], F32, tag="ps", name="ps")
                    for t in range(KT):
                        nc.tensor.matmul(
                            ps[:],
                            xk[:, st // 4, t, (st % 4) * P:(st % 4 + 1) * P],
                            A[:, t, qh * QH:(qh + 1) * QH],
                            start=(t == 0), stop=(t == KT - 1),
                        )
                    nc.scalar.activation(
                        eT[:, st, qh * QH:(qh + 1) * QH], ps[:], AF.Exp)
            rec32s = []
            for qh in range(NQ // QH):
                prs = psr.tile([1, QH], F32, tag="prs", name="prs")
                for st in range(ST):
                    nc.tensor.matmul(
                        prs[:], ones_b[:], eT[:, st, qh * QH:(qh + 1) * QH],
                        start=(st == 0), stop=(st == ST - 1))
                rec32 = miscp.tile([1, QH], F32, tag=f"rec32{qh}", name="rec32")
                nc.vector.reciprocal(rec32[:], prs[:])
                rec32s.append(rec32)
            scr_es.close()

            # attn @ V in j-groups of 3/3/1/1 (6 PSUM banks max, small tail)
            with (
                tc.tile_pool(name="pso", bufs=1, space="PSUM") as pso,
                tc.tile_pool(name="pst", bufs=1, space="PSUM") as pst,
            ):
                rcs = []
                groups = [(0, 1), (2, 3), (4, 5), (6,), (7,)]
                for gi, js in enumerate(groups):
                    pos = [
                        pso.tile([P, DVC], F32, tag=f"po{u}", name="po")
                        for u in range(len(js) * (D // DVC))
                    ]
                    for ji, j in enumerate(js):
                        for dv in range(D // DVC):
                            for st in range(ST):
                                nc.tensor.matmul(
                                    pos[ji * (D // DVC) + dv][:],
                                    eT[:, st, j * P:(j + 1) * P],
                                    V[:, st, dv * DVC:(dv + 1) * DVC],
                                    start=(st == 0), stop=(st == ST - 1),
                                )
                    if gi == 0:
                        # emitted after a dense MM batch so the ACT->DVE->PE
                        # reciprocal/transpose chain hides under the matmuls
                        for j in range(NQ // P):
                            qh, jq = divmod(j, QH // P)
                            pt = pst.tile([P, 1], F32, tag="pt", name="pt")
                            nc.tensor.transpose(
                                pt[:], rec32s[qh][:, jq * P:(jq + 1) * P],
                                ident[:])
                            rc = miscp.tile([P, 1], F32, tag=f"rc{j}", name="rc")
                            # 1/sqrt(d_k) is folded into ones=32 upstream
                            nc.vector.tensor_copy(rc[:], pt[:])
                            rcs.append(rc)
                    # last two groups evict in 256-col pieces so the
                    # ACT->DVE->DMA chain pipelines and the final DMA is
                    # small - shrinks the post-last-matmul tail
                    ev = 2 if gi >= len(groups) - 2 else 1
                    for ji, j in enumerate(js):
                        for dv in range(D // DVC):
                            po = pos[ji * (D // DVC) + dv]
                            for h2 in range(ev):
                                w2 = DVC // ev
                                osb = outp.tile([P, w2], F32,
                                                tag="osb", name="osb")
                                nc.scalar.activation(
                                    osb[:], po[:, h2 * w2:(h2 + 1) * w2],
                                    AF.Copy, scale=rcs[j][:])
                                nc.vector.tensor_tensor(
                                    osb[:], osb[:],
                                    bvb_sb[:, dv * DVC + h2 * w2:
                                           dv * DVC + (h2 + 1) * w2],
                                    op=mybir.AluOpType.add,
                                )
                                nc.scalar.dma_start(
                                    o_d.ap()[j * P:(j + 1) * P,
                                             dv * DVC + h2 * w2:
                                             dv * DVC + (h2 + 1) * w2],
                                    osb[:],
                                )
            psr_es.close()
            etp_es.close()
            vp_es.close()
            dram_es.close()
    nc.compile()
    return nc


def _get_nc():
    if "nc" not in _CACHE:
        _CACHE["nc"] = _build()
    return _CACHE["nc"]


def _preround(a, bits=13):
    # round mantissa to `bits` explicit bits (round-to-nearest), matching
    # the DVE f32->f32r rounding so raw DMA into f32r tiles is faithful
    u = np.ascontiguousarray(a, dtype=np.float32).view(np.uint32)
    shift = 23 - bits
    add = np.uint32(1 << (shift - 1))
    u = ((u.astype(np.uint64) + add) >> shift << shift).astype(np.uint32)
    return np.ascontiguousarray(u.view(np.float32))


def _in_maps(x, Wq, bq, Wk, bk, Wv, bv):
    import ml_dtypes
    def _stage(w):
        # [D, N] -> [128, N//512, 8, 512]: per-partition contiguous chunks
        return np.ascontiguousarray(
            w.reshape(KT, P, -1, QH).transpose(1, 2, 0, 3))

    M = _stage(_preround(
        np.asarray(Wq, np.float64) @ np.asarray(Wk, np.float64).T))
    c = (np.asarray(Wk, np.float64) @ np.asarray(bq, np.float64)).astype(np.float32)
    ct = np.ascontiguousarray(np.reshape(c, (KT, P)).T, dtype=np.float32)
    wv = _stage(_preround(Wv))
    bvb = np.ascontiguousarray(
        np.tile(np.asarray(bv, np.float32) / 32.0, (P, 1)).astype(ml_dtypes.bfloat16))
    x = np.asarray(x, np.float32)
    xk_stage = [_stage(_preround(x[b].T)) for b in range(B)]
    maps = []
    for cidx in range(8):
        b, h = cidx // 2, cidx % 2
        # chunks 0-1: own query rows; chunks 2-5: full x, original order
        xq = _stage(_preround(x[b, h * NQ:(h + 1) * NQ].T))
        xt = np.ascontiguousarray(np.concatenate([xq, xk_stage[b]], axis=1))
        maps.append({"xt": xt, "m": M, "wv": wv, "ct": ct, "bvb": bvb})
    return maps


def _run(inputs, trace=False, tmpdir=None):
    import time

    from concourse.bass_utils import run_bass_kernel_spmd

    nc = _get_nc()
    maps = _in_maps(**inputs)
    last_err = None
    for attempt in range(3):
        try:
            res = run_bass_kernel_spmd(nc, maps, core_ids=list(range(8)),
                                       trace=trace, tmpdir=tmpdir)
            break
        except Exception as e:  # transient NRT device errors recover on retry
            last_err = e
            time.sleep(10)
    else:
        raise last_err
    out = np.empty((B, 2 * NQ, D), dtype=np.float32)
    for cidx in range(8):
        b, h = cidx // 2, cidx % 2
        out[b, h * NQ:(h + 1) * NQ, :] = res.results[cidx]["o"]
    return out, res


def kernel(**inputs):
    out, _ = _run(inputs, trace=False)
    return out


# revision 31
# speedup vs baseline: 1.1411x; 1.0507x over previous
"""AttentionBlock kernel for 8 Trainium2 NeuronCores.

Reference computation (per batch b):
    Q = x[b] @ Wq + bq            [S, D]
    K = x[b] @ Wk + bk            [S, D]
    V = x[b] @ Wv + bv            [S, D]
    scores = Q @ K^T              [S, S]   (unscaled)
    attn = softmax(scores, -1)
    out[b] = attn @ V / sqrt(D)

Key algebraic restructuring: softmax is invariant to score terms constant
along the key axis, so
    softmax(Q K^T) = softmax(A' x^T),  A' = x_q (Wq Wk^T) + 1 (Wk bq)^T
(M = Wq Wk^T and c = Wk bq are weight-only transforms, computed on host).
This removes the K projection entirely: Q+K projections (384 matmuls per
core incl. a KT DRAM staging round-trip in the first version) become one
A' = x_q M projection (128 matmuls) with c folded in as the ACT eviction
bias. bv passes through softmax (attn rows sum to 1), so V = x @ Wv
unbiased and bv/32 is added to the final output.

Sharding: 8 cores = 4 batches x 2 query-halves. Each core computes V only
for its OWN 1024 query rows (the pair's halves are complementary in
original coordinates) and the halves are exchanged with a pairwise DRAM
AllGather that overlaps the A+scores phases - no duplicated V work. The
kernel is h-agnostic: per-core inputs carry the own-query block as
chunks 0-1 and the full key sequence in ORIGINAL order as chunks 2-5, so
eT s-tiles line up with the gathered V order on every core.

Per-core dataflow (all f32r operands pre-rounded on host to 13 mantissa
bits, matching DVE f32->f32r rounding, and DMA'd straight into resident
f32r SBUF slabs in per-partition-contiguous 16KB lines - no on-device
rounding passes; all matmuls use free-dim 512 so the fp32r LDWEIGHTS
hides under the moving-operand stream):
  1. A' [dk, q] f32r: stationary M-tiles x moving xq columns (128 mm),
     ACT Identity evict with bias c. m rides the scalar ring, wv rides
     the sync ring behind xq - both doorbells ring at program top on
     otherwise-idle engine streams, first chunks split in t-halves so
     the first psum group starts as early as possible.
  2. V_own[s_own, dv] = xq-tiles^T @ Wv (128 mm), evicted bf16, staged to
     DRAM, AllGather -> V full [S, D] bf16, read back during scores.
  3. scoresT[s-tile, q 512] = xk-tile^T @ A'-block in PSUM (256 mm); Exp
     evict to eT bf16. No max subtraction: max score ~69 stays inside
     f32/bf16 range and softmax is shift-invariant.
  4. rowsum[1, q] via ones(=32)^T @ eT (32 mm; folds the 1/sqrt(d_k));
     reciprocal on DVE; PE-transposed to per-partition [128,1].
  5. attn-output psum[q-tile, dv] = eT^T @ V over 16 s-tiles (256 mm);
     j-groups of 2/2/2/1/1 within 4 PSUM banks while the rowsum banks
     (psr) stay allocated so no bank aliases a late reciprocal; evicted
     with ACT scale=recip and a DVE +bv/32 add.
"""
import sys
from contextlib import ExitStack

sys.path.insert(0, "/opt/trn_rl_repo")

import numpy as np

P = 128
D = 1024            # d_in = d_k = d_v
S = 2048            # kv sequence per core (full batch seq)
NQ = 1024           # query rows per core
B = 4
KT = D // P         # 8 contraction tiles
ST = S // P         # 16 s tiles
QH = 512            # free-dim chunk (fp32r moving-operand limit)
DVC = 512           # dv chunk width

_CACHE = {}


def _build():
    import concourse.bacc as bacc
    import concourse.mybir as mybir
    import concourse.tile as tile

    F32 = mybir.dt.float32
    F32R = mybir.dt.float32r
    BF16 = mybir.dt.bfloat16
    AF = mybir.ActivationFunctionType

    nc = bacc.Bacc("TRN2", target_bir_lowering=False, debug=False, num_devices=8)

    # inputs staged on host in SBUF layout [part, chunk, t, col] so every
    # chunk DMA is 128 descriptors of contiguous 16KB per partition
    xt_d = nc.dram_tensor("xt", [P, 2 + S // QH, KT, QH], F32R, kind="ExternalInput")
    m_d = nc.dram_tensor("m", [P, D // QH, KT, QH], F32R, kind="ExternalInput")
    wv_d = nc.dram_tensor("wv", [P, D // QH, KT, QH], F32R, kind="ExternalInput")
    ct_d = nc.dram_tensor("ct", [P, KT], F32, kind="ExternalInput")
    bvb_d = nc.dram_tensor("bvb", [P, D], mybir.dt.bfloat16, kind="ExternalInput")
    o_d = nc.dram_tensor("o", [NQ, D], F32, kind="ExternalOutput")

    with tile.TileContext(nc) as tc:
        with (
            tc.tile_pool(name="const", bufs=1) as constp,
            tc.tile_pool(name="xrp", bufs=1) as xrp,
            tc.tile_pool(name="ap", bufs=1) as ap_pool,
            tc.tile_pool(name="misc", bufs=1) as miscp,
            tc.tile_pool(name="outp", bufs=4) as outp,
        ):
            ct_sb = constp.tile([P, KT], F32)
            bvb_sb = constp.tile([P, D], BF16)
            ones_f = constp.tile([P, 1], F32)
            nc.vector.memset(ones_f[:], 32.0)
            ones_b = constp.tile([P, 1], BF16)
            nc.vector.tensor_copy(ones_b[:], ones_f[:])
            ident = constp.tile([1, 1], F32)
            nc.vector.memset(ident[:], 1.0)

            xk = xrp.tile([P, S // QH, KT, QH], F32R)  # keys, original order
            A = ap_pool.tile([P, KT, NQ], F32R)    # [dk%128, dk//128, q]

            xq_es = ExitStack()
            xqp = xq_es.enter_context(tc.tile_pool(name="xqp", bufs=1))
            xq = xqp.tile([P, NQ // QH, KT, QH], F32R)  # own query rows
            # sync ring: query chunks first (phase A + V stationaries);
            # chunk 0 in t-halves so the first psum group starts sooner
            nc.sync.dma_start(xq[:, 0, 0:4], xt_d.ap()[:, 0, 0:4])
            nc.sync.dma_start(xq[:, 0, 4:8], xt_d.ap()[:, 0, 4:8])
            nc.sync.dma_start(xq[:, 1], xt_d.ap()[:, 1])

            wvp_es = ExitStack()
            wvp = wvp_es.enter_context(tc.tile_pool(name="wvp", bufs=1))
            proj_es = ExitStack()
            mwp = proj_es.enter_context(tc.tile_pool(name="mw", bufs=1))
            ppp_es = ExitStack()
            ppp = ppp_es.enter_context(
                tc.tile_pool(name="pp", bufs=4, space="PSUM"))

            wv_sb = wvp.tile([P, D // QH, KT, QH], F32R)
            m_sb = mwp.tile([P, D // QH, KT, QH], F32R)
            # scalar ring: ct then m (phase A weights); wv rides the sync
            # ring behind the query chunks so both phases' weights have
            # their doorbells rung at program top on otherwise-idle streams
            nc.scalar.dma_start(ct_sb[:], ct_d.ap())
            nc.scalar.dma_start(m_sb[:, 0, 0:4], m_d.ap()[:, 0, 0:4])
            nc.scalar.dma_start(m_sb[:, 0, 4:8], m_d.ap()[:, 0, 4:8])
            nc.scalar.dma_start(m_sb[:, 1], m_d.ap()[:, 1])
            for c in range(D // QH):
                nc.sync.dma_start(wv_sb[:, c], wv_d.ap()[:, c])
            for c in range(S // QH):
                nc.sync.dma_start(xk[:, c], xt_d.ap()[:, 2 + c])
            nc.scalar.dma_start(bvb_sb[:], bvb_d.ap())

            # ---- A' = x_q @ M + c (ACT bias), 128 matmuls ----
            # dk-halves outer so PE consumption follows the m chunk arrival
            # order (m0 serves both q-chunks before m1 is needed)
            for mh in range(2):
              for qc in range(NQ // QH):
                for dk in range(mh * 4, mh * 4 + 4):
                    ps = ppp.tile([P, QH], F32, tag="pp", name="ps")
                    for t in range(KT):
                        nc.tensor.matmul(
                            ps[:],
                            m_sb[:, dk // 4, t, (dk % 4) * P:(dk % 4 + 1) * P],
                            xq[:, qc, t],
                            start=(t == 0), stop=(t == KT - 1),
                        )
                    nc.scalar.activation(
                        A[:, dk, qc * QH:(qc + 1) * QH], ps[:],
                        AF.Identity, bias=ct_sb[:, dk:dk + 1],
                    )

            proj_es.close()                       # free M before V slabs
            # ---- V for own query rows (the pair's halves are
            # complementary in original coordinates) + pairwise AllGather,
            # which overlaps the scores phase ----
            vo_es = ExitStack()
            vop = vo_es.enter_context(
                tc.tile_pool(name="vop", bufs=1, side="right"))
            dram_es = ExitStack()
            dramp = dram_es.enter_context(
                tc.tile_pool(name="dram", bufs=1, space="DRAM"))
            Vown = vop.tile([P, ST // 2, D], BF16)  # [s%128, s//128, dv]
            v_half = dramp.tile([NQ, D], BF16)
            v_full = dramp.tile([S, D], BF16)
            for dv in range(D // DVC):
                for st in range(ST // 2):
                    ps = ppp.tile([P, DVC], F32, tag="pp", name="ps")
                    for t in range(KT):
                        nc.tensor.matmul(
                            ps[:],
                            xq[:, st // 4, t, (st % 4) * P:(st % 4 + 1) * P],
                            wv_sb[:, dv, t],
                            start=(t == 0), stop=(t == KT - 1),
                        )
                    nc.scalar.copy(Vown[:, st, dv * DVC:(dv + 1) * DVC], ps[:])
            v_half_r = v_half.rearrange("(t p) n -> p t n", p=P)
            nc.gpsimd.dma_start(v_half_r[:], Vown[:])
            nc.gpsimd.collective_compute(
                "AllGather",
                mybir.AluOpType.bypass,
                replica_groups=[[0, 1], [2, 3], [4, 5], [6, 7]],
                ins=[v_half.opt()],
                outs=[v_full.opt()],
            )

            wvp_es.close()                        # free Wv
            ppp_es.close()
            xq_es.close()                         # free query chunks
            vo_es.close()
            vp_es = ExitStack()
            vp = vp_es.enter_context(tc.tile_pool(name="vp", bufs=1, side="right"))
            V = vp.tile([P, ST, D], BF16)          # [s%128, s//128, dv]
            v_full_r = v_full.rearrange("(t p) n -> p t n", p=P)
            # readback overlaps the scores phase on the sync ring, split
            # by dv-halves so attnV's dv=0 groups can start on half the data
            for d2 in range(2):
                nc.sync.dma_start(V[:, :, d2 * DVC:(d2 + 1) * DVC],
                                  v_full_r[:, :, d2 * DVC:(d2 + 1) * DVC])

            # ---- attention ----
            etp_es = ExitStack()
            etp = etp_es.enter_context(tc.tile_pool(name="etp", bufs=1, side="right"))
            eT = etp.tile([P, ST, NQ], BF16, tag="eT", name="eT")
            psr_es = ExitStack()
            psr = psr_es.enter_context(
                tc.tile_pool(name="psr", bufs=2, space="PSUM"))
            scr_es = ExitStack()
            pss = scr_es.enter_context(
                tc.tile_pool(name="pss", bufs=2, space="PSUM"))
            # scoresT[s-tile, q] = xT-tile^T @ A'-block, accumulated over
            # dk; rowsum as one contiguous bf16 block afterwards (mixing the
            # bf16 ones-matmuls into the f32r stream costs a PE mode switch
            # every 8 matmuls)
            for st in range(ST):
                for qh in range(NQ // QH):
                    ps = pss.tile([P, QH], F32, tag="ps", name="ps")
                    for t in range(KT):
                        nc.tensor.matmul(
                            ps[:],
                            xk[:, st // 4, t, (st % 4) * P:(st % 4 + 1) * P],
                            A[:, t, qh * QH:(qh + 1) * QH],
                            start=(t == 0), stop=(t == KT - 1),
                        )
                    nc.scalar.activation(
                        eT[:, st, qh * QH:(qh + 1) * QH], ps[:], AF.Exp)
            rec32s = []
            for qh in range(NQ // QH):
                prs = psr.tile([1, QH], F32, tag="prs", name="prs")
                for st in range(ST):
                    nc.tensor.matmul(
                        prs[:], ones_b[:], eT[:, st, qh * QH:(qh + 1) * QH],
                        start=(st == 0), stop=(st == ST - 1))
                rec32 = miscp.tile([1, QH], F32, tag=f"rec32{qh}", name="rec32")
                nc.vector.reciprocal(rec32[:], prs[:])
                rec32s.append(rec32)
            scr_es.close()

            # attn @ V in j-groups of 3/3/1/1 (6 PSUM banks max, small tail)
            with (
                tc.tile_pool(name="pso", bufs=1, space="PSUM") as pso,
                tc.tile_pool(name="pst", bufs=1, space="PSUM") as pst,
            ):
                rcs = []
                # dv-major groups: dv=0 groups need only the first half of
                # the V readback, absorbing collective lateness; the final
                # groups are small to keep the post-last-matmul tail short
                groups = [([0, 1, 2, 3], 0), ([4, 5, 6, 7], 0),
                          ([0, 1, 2, 3], 1), ([4, 5], 1), ([6], 1), ([7], 1)]
                for gi, (js, dv) in enumerate(groups):
                    pos = [
                        pso.tile([P, DVC], F32, tag=f"po{u}", name="po")
                        for u in range(len(js))
                    ]
                    for ji, j in enumerate(js):
                        for st in range(ST):
                            nc.tensor.matmul(
                                pos[ji][:],
                                eT[:, st, j * P:(j + 1) * P],
                                V[:, st, dv * DVC:(dv + 1) * DVC],
                                start=(st == 0), stop=(st == ST - 1),
                            )
                    if gi == 0:
                        # emitted after a dense MM batch so the ACT->DVE->PE
                        # reciprocal/transpose chain hides under the matmuls
                        for j in range(NQ // P):
                            qh, jq = divmod(j, QH // P)
                            pt = pst.tile([P, 1], F32, tag="pt", name="pt")
                            nc.tensor.transpose(
                                pt[:], rec32s[qh][:, jq * P:(jq + 1) * P],
                                ident[:])
                            rc = miscp.tile([P, 1], F32, tag=f"rc{j}", name="rc")
                            # 1/sqrt(d_k) is folded into ones=32 upstream
                            nc.vector.tensor_copy(rc[:], pt[:])
                            rcs.append(rc)
                    # last two groups evict in 256-col pieces so the
                    # ACT->DVE->DMA chain pipelines and the final DMA is
                    # small - shrinks the post-last-matmul tail
                    ev = 2 if gi >= len(groups) - 2 else 1
                    for ji, j in enumerate(js):
                        po = pos[ji]
                        for h2 in range(ev):
                            w2 = DVC // ev
                            osb = outp.tile([P, w2], F32,
                                            tag="osb", name="osb")
                            nc.scalar.activation(
                                osb[:], po[:, h2 * w2:(h2 + 1) * w2],
                                AF.Copy, scale=rcs[j][:])
                            nc.vector.tensor_tensor(
                                osb[:], osb[:],
                                bvb_sb[:, dv * DVC + h2 * w2:
                                       dv * DVC + (h2 + 1) * w2],
                                op=mybir.AluOpType.add,
                            )
                            nc.scalar.dma_start(
                                o_d.ap()[j * P:(j + 1) * P,
                                         dv * DVC + h2 * w2:
                                         dv * DVC + (h2 + 1) * w2],
                                osb[:],
                            )
            psr_es.close()
            etp_es.close()
            vp_es.close()
            dram_es.close()
    nc.compile()
    return nc


def _get_nc():
    if "nc" not in _CACHE:
        _CACHE["nc"] = _build()
    return _CACHE["nc"]


def _preround(a, bits=13):
    # round mantissa to `bits` explicit bits (round-to-nearest), matching
    # the DVE f32->f32r rounding so raw DMA into f32r tiles is faithful
    u = np.ascontiguousarray(a, dtype=np.float32).view(np.uint32)
    shift = 23 - bits
    add = np.uint32(1 << (shift - 1))
    u = ((u.astype(np.uint64) + add) >> shift << shift).astype(np.uint32)
    return np.ascontiguousarray(u.view(np.float32))


def _in_maps(x, Wq, bq, Wk, bk, Wv, bv):
    import ml_dtypes
    def _stage(w):
        # [D, N] -> [128, N//512, 8, 512]: per-partition contiguous chunks
        return np.ascontiguousarray(
            w.reshape(KT, P, -1, QH).transpose(1, 2, 0, 3))

    M = _stage(_preround(
        np.asarray(Wq, np.float64) @ np.asarray(Wk, np.float64).T))
    c = (np.asarray(Wk, np.float64) @ np.asarray(bq, np.float64)).astype(np.float32)
    ct = np.ascontiguousarray(np.reshape(c, (KT, P)).T, dtype=np.float32)
    wv = _stage(_preround(Wv))
    bvb = np.ascontiguousarray(
        np.tile(np.asarray(bv, np.float32) / 32.0, (P, 1)).astype(ml_dtypes.bfloat16))
    x = np.asarray(x, np.float32)
    xk_stage = [_stage(_preround(x[b].T)) for b in range(B)]
    maps = []
    for cidx in range(8):
        b, h = cidx // 2, cidx % 2
        # chunks 0-1: own query rows; chunks 2-5: full x, original order
        xq = _stage(_preround(x[b, h * NQ:(h + 1) * NQ].T))
        xt = np.ascontiguousarray(np.concatenate([xq, xk_stage[b]], axis=1))
        maps.append({"xt": xt, "m": M, "wv": wv, "ct": ct, "bvb": bvb})
    return maps


def _run(inputs, trace=False, tmpdir=None):
    import time

    from concourse.bass_utils import run_bass_kernel_spmd

    nc = _get_nc()
    maps = _in_maps(**inputs)
    last_err = None
    for attempt in range(3):
        try:
            res = run_bass_kernel_spmd(nc, maps, core_ids=list(range(8)),
                                       trace=trace, tmpdir=tmpdir)
            break
        except Exception as e:  # transient NRT device errors recover on retry
            last_err = e
            time.sleep(10)
    else:
        raise last_err
    out = np.empty((B, 2 * NQ, D), dtype=np.float32)
    for cidx in range(8):
        b, h = cidx // 2, cidx % 2
        out[b, h * NQ:(h + 1) * NQ, :] = res.results[cidx]["o"]
    return out, res


def kernel(**inputs):
    out, _ = _run(inputs, trace=False)
    return out
